# revision 1
# baseline (speedup 1.0000x reference)
"""Trainium2 kernel for nn_CandidateFinder: LSH/Wu-Manber/Trie-masked top-64
candidate retrieval.

Math: for query (b,i) and key (b,j), the pair is a candidate iff
  sig-match:  sign-pattern of query_up[3,i] equals sign-pattern of key_up[3,j]
  lsh-match:  lsh_hash(query_up[b,i]) == lsh_hash(key_up[b,j])
  inserted:   prefix-6 sign patterns of query_up[0,j] and key_up[0,j] agree
and candidates are ranked by sims = query_up[b,i] . key_up[b,j] descending.

The device kernel fuses all three masks and the similarity into a single
PE matmul per (query,key) block producing
  z = C*(sig_agreement + 2*lsh_onehot_dot + 4*inserted) + sims
with C=1024.  A pair is a candidate iff z >= T (= 70656): matched pairs give
integer mask part 70*C, best non-matched 68*C, and |sims| << C.  Ordering by
z among matched pairs equals ordering by sims.  Per query row the DVE
max/max_index instruction pair extracts the top-8 (value-descending, ties by
lower index — identical to jax.lax.top_k's stable order).  Rows with more
than 8 candidates (8th value >= T) are detected and recomputed on host; for
iid-random inputs the expected candidate count per row is ~0 (an exact
64-bit sign-pattern collision is needed), so this path never triggers in
practice.
"""

import os
import sys

for _p in ("/opt/trn_rl_repo", os.path.expanduser("~/.axon_site/_ro/trn_rl_repo")):
    if os.path.isdir(_p) and _p not in sys.path:
        sys.path.insert(0, _p)

import numpy as np

B, S, D, H = 4, 4096, 64, 16
K_MAX = 64
PREFIX_LEN = 6
LSH_BUCKETS = 64
LSH_BANDWIDTH = 4.0
NEG = np.float32(-1e30)

N_CORES = 8
QN = (B * S) // N_CORES  # 2048 query rows per core
KN = S                   # 4096 keys (replicated)

C_SCALE = 1024.0
W_LSH = 2.0
W_INS = 4.0
# matched: 70*C + sims ; best unmatched: 68*C + sims ; |sims| <= ~260
THRESH = 69.0 * C_SCALE

_CACHE = {}


def _build_nc(reps=1):
    import concourse.bacc as bacc
    import concourse.mybir as mybir
    from concourse import masks
    from concourse.tile import TileContext

    dt = mybir.dt
    AF = mybir.ActivationFunctionType
    OP = mybir.AluOpType

    nc = bacc.Bacc("TRN2", target_bir_lowering=False, debug=False,
                   num_devices=N_CORES)

    qb = nc.dram_tensor("qb", [QN, D], dt.float32, kind="ExternalInput")
    q3 = nc.dram_tensor("q3", [QN, D], dt.float32, kind="ExternalInput")
    kb = nc.dram_tensor("kb", [KN, D], dt.float32, kind="ExternalInput")
    k3 = nc.dram_tensor("k3", [KN, D], dt.float32, kind="ExternalInput")
    wmq = nc.dram_tensor("wmq", [KN, PREFIX_LEN], dt.float32, kind="ExternalInput")
    wmk = nc.dram_tensor("wmk", [KN, PREFIX_LEN], dt.float32, kind="ExternalInput")
    lshw = nc.dram_tensor("lshw", [D, H], dt.float32, kind="ExternalInput")

    v8_out = nc.dram_tensor("v8", [QN, 16], dt.float32, kind="ExternalOutput")
    i8_out = nc.dram_tensor("i8", [QN, 16], dt.uint32, kind="ExternalOutput")

    MAGIC = 12582912.0  # 1.5 * 2**23 : float32 round-to-nearest-int magic
    QT = QN // 128      # 16 query tiles
    KC = KN // 128      # 32 key chunks
    QC = QN // 128      # 16 query chunks

    with TileContext(nc) as tc:
        with (
            tc.tile_pool(name="const", bufs=1) as cst,
            tc.tile_pool(name="feat", bufs=1) as feat,
            tc.tile_pool(name="hsb", bufs=6) as hsb,
            tc.tile_pool(name="eqp", bufs=2) as eqp,
            tc.tile_pool(name="sgtmp", bufs=2) as sgtmp,
            tc.tile_pool(name="prep", bufs=2, space="PSUM") as prep,
        ):
            ident = cst.tile([128, 128], dt.float32)
            masks.make_identity(nc, ident[:])
            w_sb = cst.tile([D, H], dt.float32)
            nc.sync.dma_start(w_sb[:], lshw[:])
            w_bf = cst.tile([D, H], dt.bfloat16)
            nc.scalar.activation(w_bf[:], w_sb[:], AF.Copy)
            ones_16x64 = cst.tile([H, 64], dt.float32)
            nc.vector.memset(ones_16x64[:], 1.0)
            ones6 = cst.tile([PREFIX_LEN, 1], dt.float32)
            nc.vector.memset(ones6[:], 1.0)
            iota_i = cst.tile([64, 1], dt.int32)
            nc.gpsimd.iota(iota_i[:], pattern=[[1, 1]], base=0, channel_multiplier=1)
            iota_f = cst.tile([64, 1], dt.float32)
            nc.scalar.activation(iota_f[:], iota_i[:], AF.Copy)

            # staged inputs: [128, nchunk*64]; chunk j col-block = tokens j*128..j*128+127
            kb_st = feat.tile([128, KC * D], dt.float32)
            k3_st = feat.tile([128, KC * D], dt.float32)
            qb_st = feat.tile([128, QC * D], dt.float32)
            q3_st = feat.tile([128, QC * D], dt.float32)
            wmq_st = feat.tile([128, KC * PREFIX_LEN], dt.float32)
            wmk_st = feat.tile([128, KC * PREFIX_LEN], dt.float32)

            def stage_half(dst, src, d, h, nh):
                ntok = (KC // nh) * 128 if dst in (kb_st, k3_st) else 0
                c0 = h * (ntok // 128) * d
                nc.sync.dma_start(
                    dst[:, c0:c0 + (ntok // 128) * d]
                    .rearrange("p (n d) -> p n d", d=d),
                    src[h * ntok:(h + 1) * ntok].rearrange("(n p) d -> p n d", p=128))

            def stage(dst, src, d):
                nc.sync.dma_start(dst[:].rearrange("p (n d) -> p n d", d=d),
                                  src[:].rearrange("(n p) d -> p n d", p=128))

            # persistent feature tensors
            fk1 = feat.tile([128, KN], dt.bfloat16)   # [0:64] sig(k3) ±1 | [64:128] onehot(kh)
            fk2 = feat.tile([65, KN], dt.bfloat16)    # [0:64] raw kb | [64] 4096*ins
            wq1 = feat.tile([128, QN], dt.bfloat16)   # [0:64] C*sig(q3) | [64:128] 2048*onehot(qh)
            wq2 = feat.tile([65, QN], dt.bfloat16)    # [0:64] raw qb | [64] 1.0
            kbt = feat.tile([D, KN], dt.float32)      # kb^T fp32 (lsh matmul rhs)
            qbt = feat.tile([D, QN], dt.float32)      # qb^T fp32
            sg_q0 = feat.tile([PREFIX_LEN, KN], dt.float32)
            sg_k0 = feat.tile([PREFIX_LEN, KN], dt.float32)
            v8_acc = feat.tile([128, QT * 16], dt.float32)
            i8_acc = feat.tile([128, QT * 16], dt.uint32)

            nc.gpsimd.memset(wq2[64:65, :], 1.0)

            def transpose_group(st, g):
                pt = prep.tile([D, 1024], dt.float32, tag="ps")
                for j in range(8):
                    c = g * 8 + j
                    nc.tensor.transpose(pt[:, j * 128:(j + 1) * 128],
                                        st[:, c * D:(c + 1) * D], ident[:])
                return pt

            def hash_group(xt, onehot_dst, scale2, g, floor_on_dve=False):
                cols = slice(g * 1024, (g + 1) * 1024)
                ph = prep.tile([H, 1024], dt.float32, tag="ps")
                for hh in range(2):
                    c0 = g * 1024 + hh * 512
                    nc.tensor.matmul(ph[:, hh * 512:(hh + 1) * 512], w_sb[:],
                                     xt[:, c0:c0 + 512], start=True, stop=True)
                # floor(proj/4) via round-to-nearest magic
                if floor_on_dve:
                    c1 = hsb.tile([H, 1024], dt.float32, tag="h")
                    nc.vector.tensor_scalar(c1[:], ph[:], 1.0 / LSH_BANDWIDTH, -0.5,
                                            OP.mult, OP.add)
                    c3 = hsb.tile([H, 1024], dt.float32, tag="h")
                    nc.vector.tensor_scalar(c3[:], c1[:], MAGIC, -MAGIC,
                                            OP.add, OP.add)
                else:
                    c1 = hsb.tile([H, 1024], dt.float32, tag="h")
                    nc.scalar.activation(c1[:], ph[:], AF.Copy,
                                         scale=1.0 / LSH_BANDWIDTH, bias=-0.5)
                    c2 = hsb.tile([H, 1024], dt.float32, tag="h")
                    nc.scalar.activation(c2[:], c1[:], AF.Copy, bias=MAGIC)
                    c3 = hsb.tile([H, 1024], dt.float32, tag="h")
                    nc.scalar.activation(c3[:], c2[:], AF.Copy, bias=-MAGIC)
                # fused sum+broadcast: [64, 1024] of per-token code sums
                pb = prep.tile([64, 1024], dt.float32, tag="ps")
                for hh in range(2):
                    nc.tensor.matmul(pb[:, hh * 512:(hh + 1) * 512], ones_16x64[:],
                                     c3[:, hh * 512:(hh + 1) * 512],
                                     start=True, stop=True)
                si = hsb.tile([64, 1024], dt.int32, tag="h")
                nc.scalar.activation(si[:], pb[:], AF.Copy)
                hi = hsb.tile([64, 1024], dt.int32, tag="h")
                nc.vector.tensor_scalar(hi[:], si[:], 63, None, OP.bitwise_and)
                hf = hsb.tile([64, 1024], dt.float32, tag="h")
                nc.scalar.activation(hf[:], hi[:], AF.Copy)
                if scale2 is None:
                    nc.vector.tensor_scalar(onehot_dst[:, cols], hf[:], iota_f[:],
                                            None, OP.is_equal)
                else:
                    nc.vector.tensor_scalar(onehot_dst[:, cols], hf[:], iota_f[:],
                                            scale2, OP.is_equal, OP.mult)

            def key_half_prep(h, floor_on_dve=False):
                stage_half(kb_st, kb, D, h, 2)
                stage_half(k3_st, k3, D, h, 2)
                # wu-manber prefix signs for this half
                wcols = slice(h * (KC // 2) * PREFIX_LEN,
                              (h + 1) * (KC // 2) * PREFIX_LEN)
                nc.sync.dma_start(
                    wmq_st[:, wcols].rearrange("p (n d) -> p n d", d=PREFIX_LEN),
                    wmq[h * (KN // 2):(h + 1) * (KN // 2)]
                    .rearrange("(n p) d -> p n d", p=128))
                nc.sync.dma_start(
                    wmk_st[:, wcols].rearrange("p (n d) -> p n d", d=PREFIX_LEN),
                    wmk[h * (KN // 2):(h + 1) * (KN // 2)]
                    .rearrange("(n p) d -> p n d", p=128))
                for g in (2 * h, 2 * h + 1):
                    pt = transpose_group(kb_st, g)
                    cols = slice(g * 1024, (g + 1) * 1024)
                    nc.scalar.activation(fk2[0:64, cols], pt[:], AF.Copy)
                    nc.scalar.activation(kbt[:, cols], pt[:], AF.Copy)
                for g in (2 * h, 2 * h + 1):
                    pt = transpose_group(k3_st, g)
                    cols = slice(g * 1024, (g + 1) * 1024)
                    nc.scalar.activation(fk1[0:64, cols], pt[:], AF.Sign)
                for g in (2 * h, 2 * h + 1):
                    hash_group(kbt, fk1[64:128, :], None, g, floor_on_dve)
                for g in (2 * h, 2 * h + 1):
                    ptq = prep.tile([PREFIX_LEN, 1024], dt.float32, tag="ps")
                    ptk = prep.tile([PREFIX_LEN, 1024], dt.float32, tag="ps")
                    for j in range(8):
                        c = g * 8 + j
                        nc.tensor.transpose(
                            ptq[:, j * 128:(j + 1) * 128],
                            wmq_st[:, c * PREFIX_LEN:(c + 1) * PREFIX_LEN], ident[:])
                        nc.tensor.transpose(
                            ptk[:, j * 128:(j + 1) * 128],
                            wmk_st[:, c * PREFIX_LEN:(c + 1) * PREFIX_LEN], ident[:])
                    cols = slice(g * 1024, (g + 1) * 1024)
                    nc.scalar.activation(sg_q0[:, cols], ptq[:], AF.Sign)
                    nc.scalar.activation(sg_k0[:, cols], ptk[:], AF.Sign)
                hcols = slice(h * (KN // 2), (h + 1) * (KN // 2))
                eq0 = eqp.tile([PREFIX_LEN, KN // 2], dt.float32, tag="eq0")
                nc.vector.tensor_tensor(eq0[:], sg_q0[:, hcols], sg_k0[:, hcols],
                                        OP.is_equal)
                for g in range(4):
                    gc = slice(g * 512, (g + 1) * 512)
                    kc = slice(h * (KN // 2) + g * 512, h * (KN // 2) + (g + 1) * 512)
                    pc = prep.tile([1, 512], dt.float32, tag="ps")
                    nc.tensor.matmul(pc[:], ones6[:], eq0[:, gc], start=True, stop=True)
                    nc.vector.tensor_scalar(fk2[64:65, kc], pc[:],
                                            float(PREFIX_LEN) - 0.5, W_INS * C_SCALE,
                                            OP.is_ge, OP.mult)

            def query_prep():
                stage(qb_st, qb, D)
                stage(q3_st, q3, D)
                for g in range(QN // 1024):         # qb
                    pt = transpose_group(qb_st, g)
                    cols = slice(g * 1024, (g + 1) * 1024)
                    nc.scalar.activation(wq2[0:64, cols], pt[:], AF.Copy)
                    nc.scalar.activation(qbt[:, cols], pt[:], AF.Copy)
                for g in range(QN // 1024):         # q3
                    pt = transpose_group(q3_st, g)
                    cols = slice(g * 1024, (g + 1) * 1024)
                    sg = sgtmp.tile([64, 1024], dt.float32, tag="sg")
                    nc.scalar.activation(sg[:], pt[:], AF.Sign)
                    nc.scalar.activation(wq1[0:64, cols], sg[:], AF.Copy,
                                         scale=C_SCALE)
                for g in range(QN // 1024):
                    hash_group(qbt, wq1[64:128, :], W_LSH * C_SCALE, g, True)

            with (
                tc.tile_pool(name="zsb", bufs=4) as zsb,
                tc.tile_pool(name="psz", bufs=2, space="PSUM") as psz,
            ):
                def phase_d_half(half, t0=0, t1=QT):
                    for t in range(t0, t1):
                        tcols = slice(t * 128, (t + 1) * 128)
                        z = zsb.tile([128, KN // 2], dt.float32, tag="z")
                        for p in range(2):
                            pz = psz.tile([128, 1024], dt.float32, tag="pz")
                            for n in range(2):
                                kcols = slice(half * 2048 + p * 1024 + n * 512,
                                              half * 2048 + p * 1024 + (n + 1) * 512)
                                nc.tensor.matmul(pz[:, n * 512:(n + 1) * 512],
                                                 wq1[:, tcols], fk1[:, kcols],
                                                 start=True, stop=False)
                            for n in range(2):
                                kcols = slice(half * 2048 + p * 1024 + n * 512,
                                              half * 2048 + p * 1024 + (n + 1) * 512)
                                nc.tensor.matmul(pz[:, n * 512:(n + 1) * 512],
                                                 wq2[:, tcols], fk2[:, kcols],
                                                 start=False, stop=True)
                            nc.scalar.activation(z[:, p * 1024:(p + 1) * 1024],
                                                 pz[:], AF.Copy)
                        ocols = slice(t * 16 + half * 8, t * 16 + half * 8 + 8)
                        nc.vector.max(v8_acc[:, ocols], z[:])
                        nc.vector.max_index(i8_acc[:, ocols], v8_acc[:, ocols], z[:])

                for _rep in range(reps):
                    query_prep()
                    key_half_prep(0, floor_on_dve=True)
                    phase_d_half(0, 0, 4)
                    key_half_prep(1)
                    phase_d_half(0, 4, QT)
                    phase_d_half(1)

            for ob in range(4):
                ts_ = slice(ob * 4 * 128, (ob + 1) * 4 * 128)
                cs_ = slice(ob * 4 * 16, (ob + 1) * 4 * 16)
                nc.sync.dma_start(
                    v8_out[ts_].rearrange("(t p) k -> p t k", p=128),
                    v8_acc[:, cs_].rearrange("p (t k) -> p t k", k=16))
                nc.sync.dma_start(
                    i8_out[ts_].rearrange("(t p) k -> p t k", p=128),
                    i8_acc[:, cs_].rearrange("p (t k) -> p t k", k=16))

    nc.compile()
    return nc


def _get_nc(reps=1):
    key = f"nc{reps}"
    if key not in _CACHE:
        _CACHE[key] = _build_nc(reps)
    return _CACHE[key]


def _reference_numpy(query_up, key_up, lsh_W):
    """Exact-semantics host fallback (only for >8-candidate rows; ~never)."""
    q = np.asarray(query_up, np.float32)
    k = np.asarray(key_up, np.float32)
    W = np.asarray(lsh_W, np.float32)
    qbin = (q > 0)
    kbin = (k > 0)

    def lsh_hash(x):
        proj = x.reshape(-1, D) @ W
        codes = np.floor(proj / LSH_BANDWIDTH).astype(np.int64)
        return (codes.sum(-1) % LSH_BUCKETS).reshape(B, S)

    qh = lsh_hash(q)
    kh = lsh_hash(k)
    inserted = np.all(qbin[0, :, :PREFIX_LEN] == kbin[0, :, :PREFIX_LEN], axis=-1)
    sig_match = np.all(qbin[-1][:, None, :] == kbin[-1][None, :, :], axis=-1)
    trie = sig_match & inserted[None, :]
    out = np.full((B, S, K_MAX), -1, np.int32)
    for b in range(B):
        lsh_m = qh[b][:, None] == kh[b][None, :]
        combined = lsh_m & trie
        sims = q[b] @ k[b].T
        masked = np.where(combined, sims, NEG)
        order = np.argsort(-masked, axis=-1, kind="stable")[:, :K_MAX]
        vals = np.take_along_axis(masked, order, axis=-1)
        out[b] = np.where(vals > NEG, order, -1).astype(np.int32)
    return out


def kernel(query_up, key_up, lsh_W, head_idx=0, **_):
    from concourse.bass_utils import run_bass_kernel_spmd

    q = np.ascontiguousarray(np.asarray(query_up, np.float32))
    k = np.ascontiguousarray(np.asarray(key_up, np.float32))
    W = np.ascontiguousarray(np.asarray(lsh_W, np.float32))

    wmq = np.ascontiguousarray(q[0, :, :PREFIX_LEN])
    wmk = np.ascontiguousarray(k[0, :, :PREFIX_LEN])

    in_maps = []
    for c in range(N_CORES):
        b = c // (N_CORES // B)
        r0 = (c % (N_CORES // B)) * QN
        in_maps.append({
            "qb": np.ascontiguousarray(q[b, r0:r0 + QN]),
            "q3": np.ascontiguousarray(q[B - 1, r0:r0 + QN]),
            "kb": np.ascontiguousarray(k[b]),
            "k3": np.ascontiguousarray(k[B - 1]),
            "wmq": wmq,
            "wmk": wmk,
            "lshw": W,
        })

    nc = _get_nc()
    res = run_bass_kernel_spmd(nc, in_maps, list(range(N_CORES))).results

    out = np.full((B, S, K_MAX), -1, np.int32)
    overflow = False
    for c in range(N_CORES):
        b = c // (N_CORES // B)
        r0 = (c % (N_CORES // B)) * QN
        v16 = res[c]["v8"]
        i16 = res[c]["i8"].astype(np.int32)
        i16 = i16 + (np.arange(16) // 8).astype(np.int32) * (KN // 2)
        order = np.argsort(-v16, axis=1, kind="stable")[:, :8]
        vtop = np.take_along_axis(v16, order, axis=1)
        itop = np.take_along_axis(i16, order, axis=1)
        out[b, r0:r0 + QN, :8] = np.where(vtop >= THRESH, itop, -1)
        if np.any(v16[:, 7] >= THRESH) or np.any(v16[:, 15] >= THRESH):
            overflow = True
    if overflow:
        return _reference_numpy(q, k, W)
    return out



# revision 4
# speedup vs baseline: 10.1995x; 10.1995x over previous
"""Trainium2 kernel for nn_CandidateFinder: LSH/Wu-Manber/Trie-masked top-64
candidate retrieval.

Math: candidates[b,i] is non-empty only if some key j satisfies
  trie_match[i,j]: ALL 64 sign bits of query_up[3,i] equal those of
  key_up[3,j]  (reference: agree >= D-0.5 with agree = exact bit-match
  count), AND lsh/inserted conditions.

Since trie_match gates every candidate, the kernel computes the exact
sign-agreement Gram matrix m[i,j] = sum_d sign(q3[i,d])*sign(k3[j,d])
(= 2*agree - 64; match iff m = 64) over all 4096x4096 (i,j) pairs,
sharded 512 query rows per core.  Detection per PSUM block is either a
DVE row-max or a fused Activation Relu(m-63)+accumulate (sum > 0 iff a
match exists).  If no pair matches anywhere, the output is exactly
all -1 (vals stay at NEG so reference emits -1 everywhere).  If any
pair matches -- never for iid Gaussian data, probability ~2^-64 per
pair -- the host recomputes the exact reference answer in numpy.

Exactness notes:
 - bf16 casting of inputs preserves sign (round-to-nearest cannot cross
   zero); any element that casts to exactly 0.0 would make sign() give 0
   (reference treats x==0 as a 0-bit), so the host falls back if any
   cast input is exactly zero (never happens for Gaussian data).
 - sign products are +-1, sums are exact small integers in fp32 PSUM.
"""

import os
import sys

for _p in ("/opt/trn_rl_repo", os.path.expanduser("~/.axon_site/_ro/trn_rl_repo")):
    if os.path.isdir(_p) and _p not in sys.path:
        sys.path.insert(0, _p)

import numpy as np

B, S, D, H = 4, 4096, 64, 16
K_MAX = 64
PREFIX_LEN = 6
LSH_BUCKETS = 64
LSH_BANDWIDTH = 4.0
NEG = np.float32(-1e30)

N_CORES = 8
QN = S // N_CORES       # 512 query rows (of batch 3) per core
KN = S                  # 4096 keys (replicated)

_CACHE = {}


def _build_nc():
    import concourse.bacc as bacc
    import concourse.mybir as mybir
    from concourse.tile import TileContext

    dt = mybir.dt
    AF = mybir.ActivationFunctionType

    nc = bacc.Bacc("TRN2", target_bir_lowering=False, debug=False,
                   num_devices=N_CORES)

    # host-transposed inputs: qs[d, i] = q3[i, d]; kh packs keys 0:2048 on
    # partitions 0:64 and keys 2048:4096 on partitions 64:128.
    qs = nc.dram_tensor("qs", [D, QN], dt.bfloat16, kind="ExternalInput")
    kh = nc.dram_tensor("kh", [2 * D, KN // 2], dt.bfloat16, kind="ExternalInput")
    det = nc.dram_tensor("det", [128, 36], dt.float32, kind="ExternalOutput")

    QT = QN // 128          # 4 query tiles per core

    with TileContext(nc) as tc:
        with (
            tc.tile_pool(name="sb", bufs=1) as sb,
            tc.tile_pool(name="ps", bufs=2, space="PSUM") as ps,
        ):
            khb = sb.tile([128, KN // 2], dt.bfloat16)
            qsb = sb.tile([D, QN], dt.bfloat16)
            skb = sb.tile([128, KN // 2], dt.bfloat16)
            sqw = sb.tile([128, QN], dt.bfloat16)
            dacc = sb.tile([128, 36], dt.float32)
            bias_t = sb.tile([128, 1], dt.float32)
            nc.vector.memset(bias_t[:], -63.0)

            nc.sync.dma_start(qsb[:], qs[:])
            for c in range(4):
                cs = slice(c * 512, (c + 1) * 512)
                nc.sync.dma_start(khb[:, cs], kh[:, cs])
            for c in range(4):
                cs = slice(c * 512, (c + 1) * 512)
                nc.scalar.activation(skb[:, cs], khb[:, cs], AF.Sign)
            nc.scalar.activation(sqw[0:D, :], qsb[:], AF.Sign)
            # replicate query signs to partitions 64:128 (SBUF->SBUF DMA)
            nc.sync.dma_start(sqw[D:2 * D, :], sqw[0:D, :])

            for t in range(QT):
                tcols = slice(t * 128, (t + 1) * 128)
                for h in range(2):
                    pp = slice(h * D, (h + 1) * D)
                    P = ps.tile([128, KN // 2], dt.float32, tag="p")
                    for c in range(4):
                        cs = slice(c * 512, (c + 1) * 512)
                        nc.tensor.matmul(P[:, cs], sqw[pp, tcols],
                                         skb[pp, cs], start=True, stop=True)
                    if h == 0:
                        nc.vector.max(dacc[:, t * 8:(t + 1) * 8], P[:])
                    else:
                        nc.scalar.activation(P[:], P[:], AF.Relu, bias=bias_t[:],
                                             accum_out=dacc[:, 32 + t:33 + t])

            nc.sync.dma_start(det[:], dacc[:])

    nc.compile()
    return nc


def _get_nc():
    if "nc" not in _CACHE:
        _CACHE["nc"] = _build_nc()
    return _CACHE["nc"]


def _reference_numpy(query_up, key_up, lsh_W):
    """Exact-semantics host fallback (only if a full sign match exists)."""
    q = np.asarray(query_up, np.float32)
    k = np.asarray(key_up, np.float32)
    W = np.asarray(lsh_W, np.float32)
    qbin = (q > 0)
    kbin = (k > 0)

    def lsh_hash(x):
        proj = x.reshape(-1, D) @ W
        codes = np.floor(proj / LSH_BANDWIDTH).astype(np.int64)
        return (codes.sum(-1) % LSH_BUCKETS).reshape(B, S)

    qh = lsh_hash(q)
    kh = lsh_hash(k)
    inserted = np.all(qbin[0, :, :PREFIX_LEN] == kbin[0, :, :PREFIX_LEN], axis=-1)
    sig_match = np.all(qbin[-1][:, None, :] == kbin[-1][None, :, :], axis=-1)
    trie = sig_match & inserted[None, :]
    out = np.full((B, S, K_MAX), -1, np.int32)
    for b in range(B):
        lsh_m = qh[b][:, None] == kh[b][None, :]
        combined = lsh_m & trie
        sims = q[b] @ k[b].T
        masked = np.where(combined, sims, NEG)
        order = np.argsort(-masked, axis=-1, kind="stable")[:, :K_MAX]
        vals = np.take_along_axis(masked, order, axis=-1)
        out[b] = np.where(vals > NEG, order, -1).astype(np.int32)
    return out


def kernel(query_up, key_up, lsh_W, head_idx=0, **_):
    import ml_dtypes
    from concourse.bass_utils import run_bass_kernel_spmd

    bf16 = ml_dtypes.bfloat16
    q = np.asarray(query_up, np.float32)
    k = np.asarray(key_up, np.float32)

    q3t = np.ascontiguousarray(q[B - 1].T).astype(bf16)        # [64, 4096]
    k3t = np.ascontiguousarray(k[B - 1].T).astype(bf16)        # [64, 4096]
    khp = np.ascontiguousarray(
        np.concatenate([k3t[:, :KN // 2], k3t[:, KN // 2:]], axis=0))

    # sign(0) = 0 would diverge from the reference's (x > 0) bit convention
    if (q3t == 0).any() or (khp == 0).any():
        return _reference_numpy(q, k, lsh_W)

    in_maps = []
    for c in range(N_CORES):
        in_maps.append({
            "qs": np.ascontiguousarray(q3t[:, c * QN:(c + 1) * QN]),
            "kh": khp,
        })

    nc = _get_nc()
    res = run_bass_kernel_spmd(nc, in_maps, list(range(N_CORES))).results

    for c in range(N_CORES):
        d = res[c]["det"]
        if (d[:, :32] > 62.5).any() or (d[:, 32:] > 0.5).any():
            return _reference_numpy(q, k, lsh_W)
    return np.full((B, S, K_MAX), -1, np.int32)


# revision 6
# speedup vs baseline: 11.3595x; 1.1137x over previous
"""Trainium2 kernel for nn_CandidateFinder: LSH/Wu-Manber/Trie-masked top-64
candidate retrieval.

Math: candidates[b,i] is non-empty only if some key j satisfies
  trie_match[i,j]: ALL 64 sign bits of query_up[3,i] equal those of
  key_up[3,j]  (reference: agree >= D-0.5), AND lsh/inserted conditions.

Since trie_match gates every candidate, the kernel screens with the
exact statistic t[i,j] = sum_d sign(q3[i,d]) * (k3[j,d] > 0), which
satisfies t[i,j] <= thr[i] := #{d : q3[i,d] > 0} for ALL keys, with
equality IFF the full 64-bit sign patterns agree.  thr is computed on
the host from the same bf16-cast data the device sees.  Detection per
PSUM block is either a DVE row-max (host compares against thr) or a
fused Activation Sign(t - thr + 0.5) + accumulate (sum == -N iff no
match in the block).  If no pair matches anywhere, the reference
output is exactly all -1.  If any pair matches (probability ~2^-64
per pair for iid Gaussian data), the host recomputes the exact
reference answer in numpy.

Exactness notes:
 - bf16 casting preserves sign (round-to-nearest cannot cross zero);
   any element that casts to exactly 0.0 would make sign() give 0, so
   the host falls back if any cast input is exactly zero.
 - sign/indicator products are 0/+-1; sums are exact ints in fp32 PSUM.
"""

import os
import sys

for _p in ("/opt/trn_rl_repo", os.path.expanduser("~/.axon_site/_ro/trn_rl_repo")):
    if os.path.isdir(_p) and _p not in sys.path:
        sys.path.insert(0, _p)

import numpy as np

B, S, D, H = 4, 4096, 64, 16
K_MAX = 64
PREFIX_LEN = 6
LSH_BUCKETS = 64
LSH_BANDWIDTH = 4.0
NEG = np.float32(-1e30)

N_CORES = 8
QN = S // N_CORES       # 512 query rows (of batch 3) per core
KN = S                  # 4096 keys (replicated)
QT = QN // 128          # 4 query tiles per core

_CACHE = {}


def _build_nc():
    import concourse.bacc as bacc
    import concourse.mybir as mybir
    from concourse.tile import TileContext

    dt = mybir.dt
    AF = mybir.ActivationFunctionType
    OP = mybir.AluOpType

    nc = bacc.Bacc("TRN2", target_bir_lowering=False, debug=False,
                   num_devices=N_CORES)

    # host-transposed inputs: qs[d, i] = q3[i, d]; kh packs keys 0:2048 on
    # partitions 0:64 and keys 2048:4096 on partitions 64:128.
    qs = nc.dram_tensor("qs", [D, QN], dt.bfloat16, kind="ExternalInput")
    kh = nc.dram_tensor("kh", [2 * D, KN // 2], dt.bfloat16, kind="ExternalInput")
    # bias[p, t] = 0.5 - thr[t*128 + p]  (per query-tile detection bias)
    bi = nc.dram_tensor("bi", [128, QT], dt.float32, kind="ExternalInput")
    det = nc.dram_tensor("det", [128, 18 * QT], dt.float32, kind="ExternalOutput")

    with TileContext(nc) as tc:
        with (
            tc.tile_pool(name="sb", bufs=1) as sb,
            tc.tile_pool(name="ps", bufs=4, space="PSUM") as ps,
        ):
            qsb = sb.tile([D, QN], dt.bfloat16)
            bib = sb.tile([128, QT], dt.float32)
            khb = sb.tile([128, KN // 2], dt.bfloat16)
            ukb = sb.tile([128, KN // 2], dt.bfloat16)
            sqw = sb.tile([128, QN], dt.bfloat16)
            dacc = sb.tile([128, 18 * QT], dt.float32)

            nc.sync.dma_start(qsb[:], qs[:])
            nc.sync.dma_start(bib[:], bi[:])
            for c in range(4):
                cs = slice(c * 512, (c + 1) * 512)
                nc.sync.dma_start(khb[:, cs], kh[:, cs])
            # keys as {0,1} on DVE (2x bf16 mode), queries as +-1 on Act
            for c in range(4):
                cs = slice(c * 512, (c + 1) * 512)
                nc.vector.tensor_scalar(ukb[:, cs], khb[:, cs], 0.0, None,
                                        OP.is_gt)
            nc.scalar.activation(sqw[0:D, :], qsb[:], AF.Sign)
            nc.scalar.activation(sqw[D:2 * D, :], qsb[:], AF.Sign)

            for t in range(QT):
                tcols = slice(t * 128, (t + 1) * 128)
                for b in range(4):
                    h = b // 2          # key half (partition range)
                    pp = slice(h * D, (h + 1) * D)
                    P = ps.tile([128, 1024], dt.float32, tag="p")
                    for c in range(2):
                        kc = slice((b % 2) * 1024 + c * 512,
                                   (b % 2) * 1024 + (c + 1) * 512)
                        nc.tensor.matmul(P[:, c * 512:(c + 1) * 512],
                                         sqw[pp, tcols], ukb[pp, kc],
                                         start=True, stop=True)
                    if b % 2 == 0:
                        # DVE block: row max; host compares against thr
                        nc.vector.max(dacc[:, t * 16 + h * 8:t * 16 + h * 8 + 8],
                                      P[:])
                    else:
                        # Act block: sign(t - thr + 0.5) summed; == -1024 iff
                        # no match in block
                        nc.scalar.activation(
                            P[:], P[:], AF.Sign, bias=bib[:, t:t + 1],
                            accum_out=dacc[:, 16 * QT + t * 2 + h:
                                           16 * QT + t * 2 + h + 1])

            nc.sync.dma_start(det[:], dacc[:])

    nc.compile()
    return nc


def _get_nc():
    if "nc" not in _CACHE:
        _CACHE["nc"] = _build_nc()
    return _CACHE["nc"]


def _reference_numpy(query_up, key_up, lsh_W):
    """Exact-semantics host fallback (only if a full sign match exists)."""
    q = np.asarray(query_up, np.float32)
    k = np.asarray(key_up, np.float32)
    W = np.asarray(lsh_W, np.float32)
    qbin = (q > 0)
    kbin = (k > 0)

    def lsh_hash(x):
        proj = x.reshape(-1, D) @ W
        codes = np.floor(proj / LSH_BANDWIDTH).astype(np.int64)
        return (codes.sum(-1) % LSH_BUCKETS).reshape(B, S)

    qh = lsh_hash(q)
    kh = lsh_hash(k)
    inserted = np.all(qbin[0, :, :PREFIX_LEN] == kbin[0, :, :PREFIX_LEN], axis=-1)
    sig_match = np.all(qbin[-1][:, None, :] == kbin[-1][None, :, :], axis=-1)
    trie = sig_match & inserted[None, :]
    out = np.full((B, S, K_MAX), -1, np.int32)
    for b in range(B):
        lsh_m = qh[b][:, None] == kh[b][None, :]
        combined = lsh_m & trie
        sims = q[b] @ k[b].T
        masked = np.where(combined, sims, NEG)
        order = np.argsort(-masked, axis=-1, kind="stable")[:, :K_MAX]
        vals = np.take_along_axis(masked, order, axis=-1)
        out[b] = np.where(vals > NEG, order, -1).astype(np.int32)
    return out


def kernel(query_up, key_up, lsh_W, head_idx=0, **_):
    import ml_dtypes
    from concourse.bass_utils import run_bass_kernel_spmd

    bf16 = ml_dtypes.bfloat16
    q = np.asarray(query_up, np.float32)
    k = np.asarray(key_up, np.float32)

    q3t = np.ascontiguousarray(q[B - 1].T).astype(bf16)        # [64, 4096]
    k3t = np.ascontiguousarray(k[B - 1].T).astype(bf16)        # [64, 4096]
    khp = np.ascontiguousarray(
        np.concatenate([k3t[:, :KN // 2], k3t[:, KN // 2:]], axis=0))

    # sign(0) = 0 would diverge from the reference's (x > 0) bit convention
    if (q3t == 0).any() or (khp == 0).any():
        return _reference_numpy(q, k, lsh_W)

    # thr[i] = #positive dims of query i (on the bf16 data the device sees);
    # t[i, j] == thr[i] iff full sign match, t < thr otherwise.
    thr = (q3t > 0).sum(axis=0).astype(np.float32)             # [4096]

    in_maps = []
    for c in range(N_CORES):
        tcol = thr[c * QN:(c + 1) * QN].reshape(QT, 128).T     # [128, QT]
        in_maps.append({
            "qs": np.ascontiguousarray(q3t[:, c * QN:(c + 1) * QN]),
            "kh": khp,
            "bi": np.ascontiguousarray(0.5 - tcol),
        })

    nc = _get_nc()
    res = run_bass_kernel_spmd(nc, in_maps, list(range(N_CORES))).results

    for c in range(N_CORES):
        d = res[c]["det"]
        thr_col = in_maps[c]["bi"]                             # 0.5 - thr
        for t in range(QT):
            vmax = d[:, t * 16:(t + 1) * 16].max(axis=1)       # DVE halves
            if (vmax + thr_col[:, t] > 0.0).any():
                return _reference_numpy(q, k, lsh_W)
        if (d[:, 16 * QT:] > -1024.0 + 0.5).any():
            return _reference_numpy(q, k, lsh_W)
    return np.full((B, S, K_MAX), -1, np.int32)


# revision 8
# speedup vs baseline: 12.8767x; 1.1336x over previous
"""Trainium2 kernel for nn_CandidateFinder: LSH/Wu-Manber/Trie-masked top-64
candidate retrieval.

Math: candidates[b,i] is non-empty only if some key j satisfies
  trie_match[i,j]: ALL 64 sign bits of query_up[3,i] equal those of
  key_up[3,j]  (reference: agree >= D-0.5), AND lsh/inserted conditions.

Since trie_match gates every candidate, the kernel screens with the
exact statistic t[i,j] = sum_d sign(q3[i,d]) * (k3[j,d] > 0), which
satisfies t[i,j] <= thr[i] := #{d : q3[i,d] > 0} for ALL keys, with
equality IFF the full 64-bit sign patterns agree.  thr is computed on
the host from the same bf16-cast data the device sees.  Detection per
PSUM block is either a DVE row-max (host compares against thr) or a
fused Activation Sign(t - thr + 0.5) + accumulate (sum == -N iff no
match in the block).  If no pair matches anywhere, the reference
output is exactly all -1.  If any pair matches (probability ~2^-64
per pair for iid Gaussian data), the host recomputes the exact
reference answer in numpy.

Exactness notes:
 - bf16 casting preserves sign (round-to-nearest cannot cross zero);
   any element that casts to exactly 0.0 would make sign() give 0, so
   the host falls back if any cast input is exactly zero.
 - sign/indicator products are 0/+-1; sums are exact ints in fp32 PSUM.
"""

import os
import sys

for _p in ("/opt/trn_rl_repo", os.path.expanduser("~/.axon_site/_ro/trn_rl_repo")):
    if os.path.isdir(_p) and _p not in sys.path:
        sys.path.insert(0, _p)

import numpy as np

B, S, D, H = 4, 4096, 64, 16
K_MAX = 64
PREFIX_LEN = 6
LSH_BUCKETS = 64
LSH_BANDWIDTH = 4.0
NEG = np.float32(-1e30)

N_CORES = 8
QN = S // N_CORES       # 512 query rows (of batch 3) per core
KN = S                  # 4096 keys (replicated)
QT = QN // 128          # 4 query tiles per core

_CACHE = {}


def _build_nc():
    import concourse.bacc as bacc
    import concourse.mybir as mybir
    from concourse.tile import TileContext

    dt = mybir.dt
    AF = mybir.ActivationFunctionType
    OP = mybir.AluOpType

    nc = bacc.Bacc("TRN2", target_bir_lowering=False, debug=False,
                   num_devices=N_CORES)

    # host-transposed inputs: qs[d, i] = q3[i, d]; kh packs keys 0:2048 on
    # partitions 0:64 and keys 2048:4096 on partitions 64:128.
    qs = nc.dram_tensor("qs", [D, QN], dt.bfloat16, kind="ExternalInput")
    kh = nc.dram_tensor("kh", [2 * D, KN // 2], dt.bfloat16, kind="ExternalInput")
    # bias[p, t] = 0.5 - thr[t*128 + p]  (per query-tile detection bias)
    bi = nc.dram_tensor("bi", [128, QT], dt.float32, kind="ExternalInput")
    det = nc.dram_tensor("det", [128, 18 * QT], dt.float32, kind="ExternalOutput")

    with TileContext(nc) as tc:
        with (
            tc.tile_pool(name="sb", bufs=1) as sb,
            tc.tile_pool(name="ps", bufs=4, space="PSUM") as ps,
        ):
            qsb = sb.tile([D, QN], dt.bfloat16)
            bib = sb.tile([128, QT], dt.float32)
            khb = sb.tile([128, KN // 2], dt.bfloat16)
            ukb = sb.tile([128, KN // 2], dt.bfloat16)
            sqw = sb.tile([128, QN], dt.bfloat16)
            dacc = sb.tile([128, 18 * QT], dt.float32)
            dum = sb.tile([128, 2], dt.bfloat16)

            # act-table preload: dummy Sign with no DMA deps runs at t=0 so
            # the 1283ns table load hides under the input-DMA window
            nc.gpsimd.memset(dum[:, 0:1], 0.0)
            nc.scalar.activation(dum[:, 1:2], dum[:, 0:1], AF.Sign)

            # small inputs via the gpsimd queue; key halves via the SP queue
            nc.gpsimd.dma_start(qsb[:], qs[:])
            nc.gpsimd.dma_start(bib[:], bi[:])
            for c in range(4):
                cs = slice(c * 512, (c + 1) * 512)
                nc.sync.dma_start(khb[:, cs], kh[:, cs])
            # keys as {0,1} on the (otherwise idle) gpsimd engine,
            # queries as +-1 on Act
            for c in range(4):
                cs = slice(c * 512, (c + 1) * 512)
                nc.gpsimd.tensor_scalar(ukb[:, cs], khb[:, cs], 0.0, None,
                                        OP.is_gt)
            nc.scalar.activation(sqw[0:D, :], qsb[:], AF.Sign)
            nc.scalar.activation(sqw[D:2 * D, :], qsb[:], AF.Sign)

            for t in range(QT):
                tcols = slice(t * 128, (t + 1) * 128)
                for b in range(4):
                    h = b // 2          # key half (partition range)
                    pp = slice(h * D, (h + 1) * D)
                    P = ps.tile([128, 1024], dt.float32, tag="p")
                    for c in range(2):
                        kc = slice((b % 2) * 1024 + c * 512,
                                   (b % 2) * 1024 + (c + 1) * 512)
                        nc.tensor.matmul(P[:, c * 512:(c + 1) * 512],
                                         sqw[pp, tcols], ukb[pp, kc],
                                         start=True, stop=True)
                    if b % 2 == 0:
                        # DVE block: row max; host compares against thr
                        nc.vector.max(dacc[:, t * 16 + h * 8:t * 16 + h * 8 + 8],
                                      P[:])
                    else:
                        # Act block: sign(t - thr + 0.5) summed; == -1024 iff
                        # no match in block
                        nc.scalar.activation(
                            P[:], P[:], AF.Sign, bias=bib[:, t:t + 1],
                            accum_out=dacc[:, 16 * QT + t * 2 + h:
                                           16 * QT + t * 2 + h + 1])

            nc.sync.dma_start(det[:], dacc[:])

    nc.compile()
    return nc


def _get_nc():
    if "nc" not in _CACHE:
        _CACHE["nc"] = _build_nc()
    return _CACHE["nc"]


def _reference_numpy(query_up, key_up, lsh_W):
    """Exact-semantics host fallback (only if a full sign match exists)."""
    q = np.asarray(query_up, np.float32)
    k = np.asarray(key_up, np.float32)
    W = np.asarray(lsh_W, np.float32)
    qbin = (q > 0)
    kbin = (k > 0)

    def lsh_hash(x):
        proj = x.reshape(-1, D) @ W
        codes = np.floor(proj / LSH_BANDWIDTH).astype(np.int64)
        return (codes.sum(-1) % LSH_BUCKETS).reshape(B, S)

    qh = lsh_hash(q)
    kh = lsh_hash(k)
    inserted = np.all(qbin[0, :, :PREFIX_LEN] == kbin[0, :, :PREFIX_LEN], axis=-1)
    sig_match = np.all(qbin[-1][:, None, :] == kbin[-1][None, :, :], axis=-1)
    trie = sig_match & inserted[None, :]
    out = np.full((B, S, K_MAX), -1, np.int32)
    for b in range(B):
        lsh_m = qh[b][:, None] == kh[b][None, :]
        combined = lsh_m & trie
        sims = q[b] @ k[b].T
        masked = np.where(combined, sims, NEG)
        order = np.argsort(-masked, axis=-1, kind="stable")[:, :K_MAX]
        vals = np.take_along_axis(masked, order, axis=-1)
        out[b] = np.where(vals > NEG, order, -1).astype(np.int32)
    return out


def kernel(query_up, key_up, lsh_W, head_idx=0, **_):
    import ml_dtypes
    from concourse.bass_utils import run_bass_kernel_spmd

    bf16 = ml_dtypes.bfloat16
    q = np.asarray(query_up, np.float32)
    k = np.asarray(key_up, np.float32)

    q3t = np.ascontiguousarray(q[B - 1].T).astype(bf16)        # [64, 4096]
    k3t = np.ascontiguousarray(k[B - 1].T).astype(bf16)        # [64, 4096]
    khp = np.ascontiguousarray(
        np.concatenate([k3t[:, :KN // 2], k3t[:, KN // 2:]], axis=0))

    # sign(0) = 0 would diverge from the reference's (x > 0) bit convention
    if (q3t == 0).any() or (khp == 0).any():
        return _reference_numpy(q, k, lsh_W)

    # thr[i] = #positive dims of query i (on the bf16 data the device sees);
    # t[i, j] == thr[i] iff full sign match, t < thr otherwise.
    thr = (q3t > 0).sum(axis=0).astype(np.float32)             # [4096]

    in_maps = []
    for c in range(N_CORES):
        tcol = thr[c * QN:(c + 1) * QN].reshape(QT, 128).T     # [128, QT]
        in_maps.append({
            "qs": np.ascontiguousarray(q3t[:, c * QN:(c + 1) * QN]),
            "kh": khp,
            "bi": np.ascontiguousarray(0.5 - tcol),
        })

    nc = _get_nc()
    res = run_bass_kernel_spmd(nc, in_maps, list(range(N_CORES))).results

    for c in range(N_CORES):
        d = res[c]["det"]
        thr_col = in_maps[c]["bi"]                             # 0.5 - thr
        for t in range(QT):
            vmax = d[:, t * 16:(t + 1) * 16].max(axis=1)       # DVE halves
            if (vmax + thr_col[:, t] > 0.0).any():
                return _reference_numpy(q, k, lsh_W)
        if (d[:, 16 * QT:] > -1024.0 + 0.5).any():
            return _reference_numpy(q, k, lsh_W)
    return np.full((B, S, K_MAX), -1, np.int32)
